# revision 1
# baseline (speedup 1.0000x reference)
"""MQA attention kernel for Trainium2, sharded over 8 NeuronCores.

Problem: query [1, 2048, 16, 128] f32, shared key/value [1, 2048, 128] f32,
mask [1, 16, 2048, 2048] bool (all ones -> no-op, per problem spec fill).

Sharding: tensor-parallel over heads, 2 heads per core; K/V replicated.

Per-core kernel. The engine budget per core is ~65.5k exp-elements/lane on
ScalarE (54.6us floor at 1.2GHz) and ~131k matmul cycles on the PE (54.6us
at 2.4GHz); everything is organized to keep both streams dense:

  - q axis (4096 cols = 2 heads x 2048, unit-major) is split into blocks of
    [512 x7, 256, 128, 128]; a "schunk" = (block, kv_tile) scores stripe
    S^T[kv 128, q w_b] computed by one fp16 matmul (fp32 PSUM, exact).
  - schunks are packed 1536-wide into [128, 1536] PSUM tiles (3 banks,
    double-buffered = 6 banks) and exp'd by ONE ScalarE activation per tile:
    43 activations/core instead of 80 -> saves ~7us of the ~204cyc/instr
    ScalarE overhead. fp16 P^T output to SBUF.
  - PV: out[q,0:128] = numerator, out[q,128] = softmax denominator in one
    PSUM accumulation group per 128-q chunk: lhsT = P^T chunk (stationary),
    rhs = [V | ones] (moving, fp16). PV matmuls are metered a few at a time
    after every scores matmul (gated per-schunk on the producing activation)
    so the PE never idles and never bursts ahead of ScalarE.
  - normalize with DVE reciprocal + tensor_scalar_mul while evacuating PSUM;
    stores per block.
  - ramp: 16 PE warmup matmuls on scratch SBUF raise the HAM clock while the
    first DMAs land; the ACT table load fires at queue start (no data deps).

Host side: pre-transposes Q/K (free on CPU), casts Q/K/V to fp16, appends
the ones column to V, scatters per-core inputs, gathers per-core outputs.
"""

import numpy as np

import concourse.bass as bass
import concourse.tile as tile
from concourse import bacc, mybir
from concourse.bass_utils import run_bass_kernel_spmd

N_CORES = 8
H = 16
HPC = H // N_CORES   # heads per core
Q = 2048
KV = 2048
D = 128
P = 128
NKV = KV // P        # 16 kv tiles
VA = D + 1           # V augmented with a ones column
QTOT = HPC * Q       # q columns per core (across its heads)
NCH = QTOT // P      # 32 output q-chunks per core
SCALE = float(1.0 / np.sqrt(np.float32(D)))

# q blocks; small tail blocks shrink the structural PV tail
BLOCK_W = [512] * 7 + [256, 128, 128]
BLOCK_OFF = [sum(BLOCK_W[:i]) for i in range(len(BLOCK_W))]
ACT_FD = 1536        # one activation instruction per [128, ACT_FD] PSUM tile

F32 = mybir.dt.float32
F16 = mybir.dt.float16

_CACHE = {}


def _plan():
    """Static schedule: schunks -> act groups, pv work queue."""
    schunks = []  # (b, i, w), block-major processing order
    for b, w in enumerate(BLOCK_W):
        for i in range(NKV):
            schunks.append((b, i, w))
    groups = []   # list of list of (b, i, w, off_in_tile)
    loc = {}      # (b, i) -> (g, off)
    cur, fd = [], 0
    # tiny leading groups so the exp stream starts as soon as the first
    # schunk's matmul lands, instead of waiting for a full 1536 tile
    flush_after = {0, 2}
    for k, (b, i, w) in enumerate(schunks):
        if fd + w > ACT_FD:
            groups.append(cur)
            cur, fd = [], 0
        assert fd % w == 0  # bank-straddle-free placement
        cur.append((b, i, w, fd))
        loc[(b, i)] = (len(groups), fd)
        fd += w
        if k in flush_after:
            groups.append(cur)
            cur, fd = [], 0
    groups.append(cur)
    chunks = []   # (b, jloc, global_j) 128-q output chunks
    j = 0
    for b, w in enumerate(BLOCK_W):
        for jl in range(w // P):
            chunks.append((b, jl, j))
            j += 1
    return groups, loc, chunks


def _build():
    nc = bacc.Bacc("TRN2", target_bir_lowering=False, debug=False,
                   num_devices=N_CORES)
    groups, loc, chunks = _plan()

    pre = nc.dram_tensor("pre", [P, 3 * P + 512], F16, kind="ExternalInput")
    kT = nc.dram_tensor("kT", [P, KV], F16, kind="ExternalInput")
    qT = nc.dram_tensor("qT", [P, QTOT], F16, kind="ExternalInput")
    vaug = nc.dram_tensor("vaug", [P, NKV * VA], F16, kind="ExternalInput")
    # partition-major output: o[p, j*D + d] for q-chunk j -> one contiguous
    # 512B-2KB descriptor per partition per store instead of 4x 512B ones
    o = nc.dram_tensor("o", [P, NCH * D], F32, kind="ExternalOutput")

    # qT SBUF regions (block-aligned); block 0 comes via preQ
    QREG = [(512, 1536), (1536, 4096)]

    with tile.TileContext(nc) as tc:
        with (
            tc.tile_pool(name="const", bufs=1) as const_pool,
            tc.tile_pool(name="pT", bufs=12) as pT_pool,
            tc.tile_pool(name="osb", bufs=3) as osb_pool,
            tc.tile_pool(name="recip", bufs=4) as recip_pool,
            tc.tile_pool(name="psumS", bufs=2, space="PSUM") as psumS_pool,
            tc.tile_pool(name="psumO", bufs=2, space="PSUM") as psumO_pool,
        ):
            # PE warmup: 16 matmuls (~3.4us sustained) flip the HAM clock
            # gate to 2.4GHz before the first data-dependent matmul; the
            # early DMA stalls would otherwise keep resetting the activity
            # window and the whole ramp would run at 1.2GHz
            wa = const_pool.tile([P, 256], F16)
            nc.vector.memset(wa[:], 0.0)
            wp = psumO_pool.tile([P, 256], F32, name="wp", tag="po")
            for _ in range(12):
                nc.tensor.matmul(wp[:], wa[:, 0:P], wa[:], start=True,
                                 stop=True)

            # input DMAs, ordered by first use; only three upfront — more
            # would round-robin-steal bandwidth from the act0-gating pre
            pre_sb = const_pool.tile([P, 3 * P + 512], F16)
            nc.sync.dma_start(pre_sb[:], pre.ap())
            kT_sb = const_pool.tile([P, KV], F16)
            nc.sync.dma_start(kT_sb[:, 3 * P:9 * P], kT.ap()[:, 3 * P:9 * P])
            nc.sync.dma_start(kT_sb[:, 9 * P:], kT.ap()[:, 9 * P:])
            vaug_sb = const_pool.tile([P, NKV * VA], F16)
            q_sbs = []
            for (lo, hi) in QREG:
                t = const_pool.tile([P, hi - lo], F16, name=f"q{lo}")
                q_sbs.append(t)
            # q0 / qrest are not needed until mid-ramp or later, but SDMA
            # round-robins all queued work at packet granularity and would
            # starve the urgently-needed kTa/kTb/vaug. Each is gated by a
            # 1-element DVE write that depends on an early pT tile, so its
            # descriptor generation (and transfer) starts only once the
            # ramp-critical DMAs are done (gates in the group loop below).

            def q_src(b):
                off, w = BLOCK_OFF[b], BLOCK_W[b]
                if off + w <= 512:
                    return pre_sb[:, 3 * P + off:3 * P + off + w]
                for t, (lo, hi) in zip(q_sbs, QREG):
                    if lo <= off and off + w <= hi:
                        return t[:, off - lo:off - lo + w]
                raise AssertionError

            # --- steady state ---
            pT_sbs = {}    # g -> tile
            osb_sbs = {}   # b -> tile
            po_cur = {}    # live po tiles keyed by global chunk j

            pvq = []       # flat PV work queue
            for (b, jl, j) in chunks:
                for i in range(NKV):
                    pvq.append(("mm", b, jl, j, i))
                pvq.append(("evac", b, jl, j))
                if jl == BLOCK_W[b] // P - 1:
                    pvq.append(("store", b, j))
            state = {"pos": 0, "mms": 0, "g_emitted": 0, "s": 0}

            def pv_step(op):
                kind = op[0]
                if kind == "mm":
                    _, b, jl, j, i = op
                    if i == 0:
                        po_cur[j] = psumO_pool.tile([P, VA], F32, name="po",
                                                    tag="po")
                        if b not in osb_sbs:
                            osb_sbs[b] = osb_pool.tile(
                                [P, BLOCK_W[b]], F32, name="osb", tag="osb",
                                padded_shape=[P, 512])
                    g, off = loc[(b, i)]
                    nc.tensor.matmul(
                        po_cur[j][:],
                        pT_sbs[g][:, off + jl * P:off + (jl + 1) * P],
                        vaug_sb[:, i * VA:(i + 1) * VA],
                        start=(i == 0), stop=(i == NKV - 1),
                        skip_group_check=True,
                    )
                    state["mms"] += 1
                elif kind == "evac":
                    _, b, jl, j = op
                    po = po_cur.pop(j)
                    rc = recip_pool.tile([P, 1], F32, name="rc", tag="rc")
                    nc.vector.reciprocal(rc[:], po[:, D:D + 1])
                    nc.vector.tensor_scalar_mul(
                        osb_sbs[b][:, jl * P:(jl + 1) * P], po[:, 0:D], rc[:])
                else:
                    _, b, j = op
                    w = BLOCK_W[b]
                    jlo = j - (w // P - 1)
                    nc.sync.dma_start(
                        o.ap()[:, jlo * D:(j + 1) * D],
                        osb_sbs.pop(b)[:, 0:w])

            def drain(cap=6):
                # pop PV work: mm ops are gated on the producing activation
                # having been emitted, and metered to ~4 mms per 512-wide
                # schunk (proportionally fewer for narrow ones)
                target = max(0, int(4.0 * (state["s"] - 20)))
                popped = 0
                while state["pos"] < len(pvq):
                    op = pvq[state["pos"]]
                    if op[0] == "mm":
                        _, b, jl, j, i = op
                        if loc[(b, i)][0] >= state["g_emitted"]:
                            break
                        if state["mms"] >= target or popped >= cap:
                            break
                        popped += 1
                    pv_step(op)
                    state["pos"] += 1

            NG = len(groups)
            for g, grp in enumerate(groups):
                tail = g >= NG - 3
                fd = sum(w for (_, _, w, _) in grp)
                ps = psumS_pool.tile([P, fd], F32, name="ps", tag="ps",
                                     padded_shape=[P, ACT_FD])
                for (b, i, w, off) in grp:
                    if i < 3:
                        kt = pre_sb[:, i * P:(i + 1) * P]
                    else:
                        kt = kT_sb[:, i * P:(i + 1) * P]
                    nc.tensor.matmul(ps[:, off:off + w], kt, q_src(b),
                                     start=True, stop=True,
                                     skip_group_check=True)
                    state["s"] += 1
                    if not tail:
                        drain(cap=max(2, w // 112))
                pT = pT_pool.tile([P, fd], F16, name="pT", tag="pT",
                                  padded_shape=[P, ACT_FD])
                nc.scalar.activation(pT[:], ps[:],
                                     mybir.ActivationFunctionType.Exp,
                                     scale=SCALE)
                pT_sbs[g] = pT
                state["g_emitted"] = g + 1
                if not tail:
                    drain()
                # tail groups: no PV in between — the final scores + acts
                # issue back-to-back, then the PV tail drains after
                if g == 1:
                    nc.vector.tensor_scalar_mul(vaug_sb[0:1, 0:1],
                                                pT[0:1, 0:1], 0.0)
                    nc.sync.dma_start(vaug_sb[:], vaug.ap())
                elif g == 2:
                    nc.vector.tensor_scalar_mul(q_sbs[0][0:1, 0:1],
                                                pT[0:1, 0:1], 0.0)
                    nc.sync.dma_start(q_sbs[0][:],
                                      qT.ap()[:, QREG[0][0]:QREG[0][1]])
                elif g == 6:
                    nc.vector.tensor_scalar_mul(q_sbs[1][0:1, 0:1],
                                                pT[0:1, 0:1], 0.0)
                    nc.sync.dma_start(q_sbs[1][:],
                                      qT.ap()[:, QREG[1][0]:QREG[1][1]])
            while state["pos"] < len(pvq):
                pv_step(pvq[state["pos"]])
                state["pos"] += 1
    nc.compile()
    return nc


def _get_nc():
    if "nc" not in _CACHE:
        _CACHE["nc"] = _build()
    return _CACHE["nc"]


def kernel(query_states, key_states, value_states, attention_mask):
    # mask is all-ones by problem construction -> identity; ignored.
    q = np.asarray(query_states, dtype=np.float32).reshape(Q, H, D)
    k = np.asarray(key_states, dtype=np.float32).reshape(KV, D)
    v = np.asarray(value_states, dtype=np.float32).reshape(KV, D)

    kT = np.ascontiguousarray(k.T).astype(np.float16)  # [128, KV]
    # [V | ones] in fp16, laid out [128 kv-local, NKV * 129]
    va = np.concatenate(
        [v.reshape(NKV, P, D), np.ones((NKV, P, 1), np.float32)], axis=2
    ).astype(np.float16)
    vaug = np.ascontiguousarray(va.transpose(1, 0, 2)).reshape(P, NKV * VA)

    in_maps = []
    for c in range(N_CORES):
        qTc = np.empty((P, QTOT), np.float16)
        for hh in range(HPC):
            qTc[:, hh * Q:(hh + 1) * Q] = q[:, c * HPC + hh, :].T
        pre = np.ascontiguousarray(
            np.concatenate([kT[:, 0:3 * P], qTc[:, 0:512]], axis=1))
        in_maps.append({"qT": qTc, "kT": kT, "vaug": vaug, "pre": pre})

    nc = _get_nc()
    res = run_bass_kernel_spmd(nc, in_maps, core_ids=list(range(N_CORES)))

    out = np.empty((Q, H, D), dtype=np.float32)
    for c in range(N_CORES):
        # o[p, j*D+d] -> q-major [QTOT, D] with q = j*128 + p
        oc = res.results[c]["o"].reshape(P, NCH, D).transpose(1, 0, 2)
        oc = oc.reshape(QTOT, D)
        for hh in range(HPC):
            out[:, c * HPC + hh, :] = oc[hh * Q:(hh + 1) * Q]
    return out.reshape(1, Q, H, D)



# revision 2
# speedup vs baseline: 1.0175x; 1.0175x over previous
"""MQA attention kernel for Trainium2, sharded over 8 NeuronCores.

Problem: query [1, 2048, 16, 128] f32, shared key/value [1, 2048, 128] f32,
mask [1, 16, 2048, 2048] bool (all ones -> no-op, per problem spec fill).

Sharding: tensor-parallel over heads, 2 heads per core; K/V replicated.

Per-core roofline: 65536 exp-elements/lane on ScalarE (54.6us at 1.2GHz) and
~131.6k matmul stream cycles on the PE (54.8us at 2.4GHz) -- co-critical.
The schedule keeps the ScalarE exp stream dense from ~9us to the end:

  - scores are computed as 128-col "rects" S^T[kv 128, q 128] (one fp16
    matmul each; 128-col matmuls sustain ~60ns incl hidden LDWEIGHTS).
    Rect order is chosen for DMA arrival + PV readiness:
      phase A: q[0:512] x kv tiles 0-7   (only needs the two upfront DMAs)
      phase B: q[0:512] x kv tiles 8-15  (needs kT tail, arrives ~12.4us)
      phase C: per 128-q chunk, all 16 kv tiles (chunk becomes PV-ready
               ~1.7us after its columns are exp'd -> small structural tail)
  - rects pack into [128, <=1536] PSUM tiles (3 banks, double-buffered);
    ONE ScalarE Exp per tile; ramp groups are small ([1,2,3,6] rects) so
    the exp stream starts as soon as the first 128KB DMA lands.
  - PV: out[q,0:128]+denominator in one PSUM accumulation group per 128-q
    chunk (lhsT = pT piece stationary, rhs = [V | ones] moving). PV pops
    are gated LAG=2 groups behind the exp stream: backlog drains into PE
    slack and overlaps the final activations.
  - input DMAs: two ungated upfront (act0's data alone first), later waves
    gated on the FIRST scores matmul's PSUM write via a 1-elem DVE read,
    hiding the ~1.5us HWDGE first-byte latency without round-robin
    stealing bandwidth from the ramp-critical transfers.
  - ~20 PE warmup matmuls raise the HAM clock gate during the preamble.

Host side: pre-transposes Q/K (free on CPU), casts to fp16, appends the
ones column to V, scatters per-core inputs, gathers per-core outputs.
"""

import numpy as np

import concourse.bass as bass
import concourse.tile as tile
from concourse import bacc, mybir
from concourse.bass_utils import run_bass_kernel_spmd

N_CORES = 8
H = 16
HPC = H // N_CORES   # heads per core
Q = 2048
KV = 2048
D = 128
P = 128
NKV = KV // P        # 16 kv tiles
VA = D + 1           # V augmented with a ones column
QTOT = HPC * Q       # q columns per core (across its heads)
NCH = QTOT // P      # 32 output q-chunks per core
SCALE = float(1.0 / np.sqrt(np.float32(D)))

ACT_FD = 1536        # one activation instruction per [128, ACT_FD] PSUM tile
RAMP_PACK = [1, 2, 3, 6]   # rects per act group during the ramp
LAG = 2              # PV pops trail the exp stream by this many groups

# store blocks, in 128-q chunks: shrink toward the end for a short tail
STORE_BLK = [4, 4, 4, 4, 4, 4, 4, 2, 1, 1]

F32 = mybir.dt.float32
F16 = mybir.dt.float16

_CACHE = {}


def _plan():
    """Static schedule.

    Returns:
      groups: list of act groups; each is a list of rects (qb, i, off) with
              off the rect's column offset inside the group's PSUM tile.
      loc:    (i, chunk j) -> (g, off) location of that pT piece.
      """
    rects = []  # (qb, i)
    # phase A: q[0:512] x i 0..7, qb-major (i-minor) to match DMA arrival
    for qb in range(0, 512, P):
        for i in range(8):
            rects.append((qb, i))
    # phase B: q[0:512] x i 8..15
    for qb in range(0, 512, P):
        for i in range(8, 16):
            rects.append((qb, i))
    # phase C: remaining q, full i sweep per 128-q chunk
    for qb in range(512, QTOT, P):
        for i in range(NKV):
            rects.append((qb, i))
    assert len(rects) == (QTOT // P) * NKV

    groups = []
    loc = {}
    k = 0
    ramp = list(RAMP_PACK)
    while k < len(rects):
        n = ramp.pop(0) if ramp else ACT_FD // P
        n = min(n, len(rects) - k)
        grp = []
        for m in range(n):
            qb, i = rects[k + m]
            off = m * P
            grp.append((qb, i, off))
            loc[(i, qb // P)] = (len(groups), off)
        groups.append(grp)
        k += n
    return groups, loc


def _build():
    nc = bacc.Bacc("TRN2", target_bir_lowering=False, debug=False,
                   num_devices=N_CORES)
    groups, loc = _plan()
    NG = len(groups)

    kT = nc.dram_tensor("kT", [P, KV], F16, kind="ExternalInput")
    qT = nc.dram_tensor("qT", [P, QTOT], F16, kind="ExternalInput")
    vaug = nc.dram_tensor("vaug", [P, NKV * VA], F16, kind="ExternalInput")
    # partition-major output: o[p, j*D + d] for q-chunk j
    o = nc.dram_tensor("o", [P, NCH * D], F32, kind="ExternalOutput")

    with tile.TileContext(nc) as tc:
        with (
            tc.tile_pool(name="const", bufs=1) as const_pool,
            tc.tile_pool(name="pT", bufs=12) as pT_pool,
            tc.tile_pool(name="osb", bufs=3) as osb_pool,
            tc.tile_pool(name="recip", bufs=4) as recip_pool,
            tc.tile_pool(name="psumS", bufs=2, space="PSUM") as psumS_pool,
            tc.tile_pool(name="psumO", bufs=2, space="PSUM") as psumO_pool,
        ):
            # PE warmup: flips the HAM clock gate to 2.4GHz while the first
            # DMAs are in flight; sized to cover until the first data
            # matmul can run (~2us at the throttled 1.2GHz clock)
            wa = const_pool.tile([P, 256], F16)
            nc.gpsimd.memset(wa[:], 0.0)
            wp = psumO_pool.tile([P, 256], F32, name="wp", tag="po")
            for _ in range(20):
                nc.tensor.matmul(wp[:], wa[:, 0:P], wa[:], start=True,
                                 stop=True)

            kT_sb = const_pool.tile([P, KV], F16)
            qT_sb = const_pool.tile([P, QTOT], F16)
            vaug_sb = const_pool.tile([P, NKV * VA], F16)

            # upfront DMAs: exactly what the ramp needs, smallest first
            # W0a: kT tiles 0-2 + q[0:128]  (act0's data, 128KB)
            nc.sync.dma_start(kT_sb[:, 0:3 * P], kT.ap()[:, 0:3 * P])
            nc.sync.dma_start(qT_sb[:, 0:P], qT.ap()[:, 0:P])
            # W0b: kT tiles 3-7 + q[128:512] (256KB)
            nc.sync.dma_start(kT_sb[:, 3 * P:8 * P], kT.ap()[:, 3 * P:8 * P])
            nc.sync.dma_start(qT_sb[:, P:512], qT.ap()[:, P:512])

            # later waves, gated below on early scores-matmul PSUM writes:
            #   W1 (after mm0): kT tiles 8-15 + vaug (whole)
            #   W2 (after g1 mms): q[512:1024]
            #   W3 (after g3 mms): q[1024:2048]
            #   W4 (after g5 mms): q[2048:4096]
            waves = {
                0: [(kT_sb, kT, 8 * P, 16 * P), (vaug_sb, vaug, 0, NKV * VA)],
                1: [(qT_sb, qT, 512, 1024)],
                3: [(qT_sb, qT, 1024, 2048)],
                5: [(qT_sb, qT, 2048, 4096)],
            }

            # --- steady state ---
            pT_sbs = {}    # g -> pT tile
            osb_sbs = {}   # block -> tile
            po_cur = {}    # live po tiles keyed by chunk j

            # PV work queue: chunk-major, with evac + store milestones
            blk_of = {}    # chunk j -> (block, jlo, w)
            j0 = 0
            for b, nchunks in enumerate(STORE_BLK):
                for jl in range(nchunks):
                    blk_of[j0 + jl] = (b, j0, nchunks * P)
                j0 += nchunks
            pvq = []
            for j in range(NCH):
                for i in range(NKV):
                    pvq.append(("mm", j, i))
                pvq.append(("evac", j))
                b, jlo, w = blk_of[j]
                if j == jlo + w // P - 1:
                    pvq.append(("store", j))
            state = {"pos": 0, "g_emitted": 0}

            def pv_step(op):
                kind = op[0]
                if kind == "mm":
                    _, j, i = op
                    if i == 0:
                        po_cur[j] = psumO_pool.tile([P, VA], F32, name="po",
                                                    tag="po")
                        b, jlo, w = blk_of[j]
                        if b not in osb_sbs:
                            osb_sbs[b] = osb_pool.tile(
                                [P, w], F32, name="osb", tag="osb",
                                padded_shape=[P, 512])
                    g, off = loc[(i, j)]
                    nc.tensor.matmul(
                        po_cur[j][:],
                        pT_sbs[g][:, off:off + P],
                        vaug_sb[:, i * VA:(i + 1) * VA],
                        start=(i == 0), stop=(i == NKV - 1),
                        skip_group_check=True,
                    )
                elif kind == "evac":
                    _, j = op
                    po = po_cur.pop(j)
                    b, jlo, w = blk_of[j]
                    rc = recip_pool.tile([P, 1], F32, name="rc", tag="rc")
                    nc.vector.reciprocal(rc[:], po[:, D:D + 1])
                    nc.vector.tensor_scalar_mul(
                        osb_sbs[b][:, (j - jlo) * P:(j - jlo + 1) * P],
                        po[:, 0:D], rc[:])
                else:
                    _, j = op
                    b, jlo, w = blk_of[j]
                    nc.sync.dma_start(
                        o.ap()[:, jlo * D:jlo * D + w * (D // P)],
                        osb_sbs.pop(b)[:, 0:w])

            def ready(op):
                if op[0] != "mm":
                    return True
                _, j, i = op
                return loc[(i, j)][0] + LAG < state["g_emitted"]

            def drain(cap):
                popped = 0
                while state["pos"] < len(pvq) and popped < cap:
                    op = pvq[state["pos"]]
                    if not ready(op):
                        break
                    pv_step(op)
                    state["pos"] += 1
                    if op[0] == "mm":
                        popped += 1

            for g, grp in enumerate(groups):
                fd = len(grp) * P
                ps = psumS_pool.tile([P, fd], F32, name="ps", tag="ps",
                                     padded_shape=[P, ACT_FD])
                for (qb, i, off) in grp:
                    nc.tensor.matmul(ps[:, off:off + P],
                                     kT_sb[:, i * P:(i + 1) * P],
                                     qT_sb[:, qb:qb + P],
                                     start=True, stop=True,
                                     skip_group_check=True)
                    drain(cap=3)
                pT = pT_pool.tile([P, fd], F16, name="pT", tag="pT",
                                  padded_shape=[P, ACT_FD])
                nc.scalar.activation(pT[:], ps[:],
                                     mybir.ActivationFunctionType.Exp,
                                     scale=SCALE)
                pT_sbs[g] = pT
                state["g_emitted"] = g + 1
                drain(cap=8)
                if g in waves:
                    for (sb, dram, lo, hi) in waves[g]:
                        # 1-elem DVE write gated on this group's scores
                        # PSUM -> orders the DMA ~0.1us after the matmul,
                        # well before the act completes
                        nc.vector.tensor_scalar_mul(sb[0:1, lo:lo + 1],
                                                    ps[0:1, 0:1], 0.0)
                        nc.sync.dma_start(sb[:, lo:hi], dram.ap()[:, lo:hi])
            while state["pos"] < len(pvq):
                pv_step(pvq[state["pos"]])
                state["pos"] += 1
    nc.compile()
    return nc


def _get_nc():
    if "nc" not in _CACHE:
        _CACHE["nc"] = _build()
    return _CACHE["nc"]


def kernel(query_states, key_states, value_states, attention_mask):
    # mask is all-ones by problem construction -> identity; ignored.
    q = np.asarray(query_states, dtype=np.float32).reshape(Q, H, D)
    k = np.asarray(key_states, dtype=np.float32).reshape(KV, D)
    v = np.asarray(value_states, dtype=np.float32).reshape(KV, D)

    kT = np.ascontiguousarray(k.T).astype(np.float16)  # [128, KV]
    # [V | ones] in fp16, laid out [128 kv-local, NKV * 129]
    va = np.concatenate(
        [v.reshape(NKV, P, D), np.ones((NKV, P, 1), np.float32)], axis=2
    ).astype(np.float16)
    vaug = np.ascontiguousarray(va.transpose(1, 0, 2)).reshape(P, NKV * VA)

    in_maps = []
    for c in range(N_CORES):
        qTc = np.empty((P, QTOT), np.float16)
        for hh in range(HPC):
            qTc[:, hh * Q:(hh + 1) * Q] = q[:, c * HPC + hh, :].T
        in_maps.append({"qT": qTc, "kT": kT, "vaug": vaug})

    nc = _get_nc()
    res = run_bass_kernel_spmd(nc, in_maps, core_ids=list(range(N_CORES)))

    out = np.empty((Q, H, D), dtype=np.float32)
    for c in range(N_CORES):
        # o[p, j*D+d] -> q-major [QTOT, D] with q = j*128 + p
        oc = res.results[c]["o"].reshape(P, NCH, D).transpose(1, 0, 2)
        oc = oc.reshape(QTOT, D)
        for hh in range(HPC):
            out[:, c * HPC + hh, :] = oc[hh * Q:(hh + 1) * Q]
    return out.reshape(1, Q, H, D)
